# revision 40
# baseline (speedup 1.0000x reference)
"""Trainium2 Bass kernel: 10-layer LSTM autoencoder (5-layer encoder H=256 +
5-layer decoder hidden=1280), T=128, B=256, D=1280.

Strategy: pure data-parallel over batch (b=32 per core, 8 cores, no
collectives).  All matmuls run with the small activation tile (32 cols) as the
PE stationary operand, replicated into the four 32-wide column groups of the
128x128 array via tile_position; each column group streams one gate-quarter of
the weights, so the array is fully utilized despite batch=32.  Gate quarters
are ordered [i, f, o, g] so a single sigmoid covers PSUM partitions 0..95.
Weights/activations are bf16 (fp32 PSUM accumulate), biases are folded into
the precomputed input projection xg.  The xg contribution enters the gates
PSUM through an identity matmul (PE) instead of a DVE add, keeping the
per-step serial chain short.

Time steps are processed in PAIRS: every sequence tensor is laid out
[T/2, 128, 2*W] so one dynamic DMA moves two steps, every For_i covers two
steps, and the h^T state is split per k-chunk into small tiles so the next
step's matmuls can start as soon as the first hidden chunk of h^T lands
(pipelining the elementwise chain under the PE).

The recurrence loop for layer l is fused with the input projection of layer
l+1 whenever both weight sets fit in SBUF (all encoder boundaries + dec0);
decoder-to-decoder boundaries run as separate projection passes through DRAM.
Engine registers are the scarce resource (2 per For_i per engine + 2 per
dynamic-DMA site, and only snapped IV copies can be safely recycled), so
dynamic DMAs alternate between the two HWDGE-capable queues (SP, Act).
"""
import sys
import os
import numpy as np
import ml_dtypes

sys.path.insert(0, "/opt/trn_rl_repo")

T, B, D, H, L = 128, 256, 1280, 256, 5
NCORES = 8
BL = B // NCORES  # 32 batch per core
TP = T // 2       # time pairs

LAYERS = [(D, H)] + [(H, H)] * 4 + [(H, D)] + [(D, D)] * 4

BF = ml_dtypes.bfloat16


def _nchunks(hid):
    out = []
    n0 = 0
    while n0 < hid:
        out.append((n0, min(512, hid - n0)))
        n0 += 512
    return out


def _split_waits(nc, mybir):
    """Walrus's per-opcode instruction structs hold very few sync-wait slots
    (LDWEIGHTS holds one).  Tile can emit many waits on one instruction, so
    dedupe same-semaphore waits (keep max threshold) and spill all but one
    onto dedicated NoOps placed immediately before the instruction."""
    counter = [0]
    for fn in nc.m.functions:
        for bb in fn.blocks:
            out = []
            for ins in bb.instructions:
                si = getattr(ins, "sync_info", None)
                if si is not None and si.on_wait and len(si.on_wait) > 1:
                    merged = {}
                    order = []
                    for w in si.on_wait:
                        key = (w.sync_type, w.id, w.wait_mode, str(w.wait_reg))
                        if key in merged:
                            old = merged[key]
                            if (w.wait_value is not None and old.wait_value is not None
                                    and w.wait_value > old.wait_value):
                                merged[key] = w
                        else:
                            merged[key] = w
                            order.append(key)
                    waits = [merged[k] for k in order]
                    for w in waits[:-1]:
                        counter[0] += 1
                        out.append(mybir.InstNoOp(
                            name=f"waitspill-{counter[0]}",
                            engine=ins.engine,
                            ins=[], outs=[],
                            sync_info=mybir.SyncInfo(on_wait=[w], on_update=[]),
                        ))
                    ins.sync_info = mybir.SyncInfo(on_wait=[waits[-1]], on_update=si.on_update)
                out.append(ins)
            bb.instructions = out


def build_graph(staggered=None):
    from concourse import bass, tile
    try:
        from concourse import tile_utils
        tile_utils.max_sbuf_usage = 206 * 1024
    except Exception:
        pass
    import concourse.mybir as mybir

    if staggered is None:
        staggered = os.environ.get("STAGGERED", "0") == "1"

    f32 = mybir.dt.float32
    bf16 = mybir.dt.bfloat16
    Sigmoid = mybir.ActivationFunctionType.Sigmoid
    Tanh = mybir.ActivationFunctionType.Tanh

    nc = bass.Bass()

    xT = nc.declare_dram_parameter("xT", [TP, 128, 2 * (D // 128) * BL], bf16, isOutput=False)
    wih, whh, bias = [], [], []
    for li, (IN, HID) in enumerate(LAYERS):
        wih.append(nc.declare_dram_parameter(f"wih{li}", [4, IN // 128, 128, HID], bf16, isOutput=False))
        whh.append(nc.declare_dram_parameter(f"whh{li}", [4, HID // 128, 128, HID], bf16, isOutput=False))
        bias.append(nc.declare_dram_parameter(f"bias{li}", [128, HID], f32, isOutput=False))
    ident_dram = nc.declare_dram_parameter("ident32", [32, 32], f32, isOutput=False)
    ident128_dram = nc.declare_dram_parameter("ident128", [128, 128], bf16, isOutput=False)
    enc_out = nc.declare_dram_parameter("enc_out", [TP, 128, 2 * (H // 128) * BL], bf16, isOutput=True)
    dec_out = nc.declare_dram_parameter("dec_out", [TP, 128, 2 * (D // 128) * BL], bf16, isOutput=True)

    xg_buf = {
        hid: [nc.dram_tensor(f"xg{hid}_{i}", [TP, 128, 2 * hid], bf16) for i in range(2)]
        for hid in (H, D)
    }
    hseq_buf = {li: nc.dram_tensor(f"hseq{li}", [TP, 128, 2 * (D // 128) * BL], bf16)
                for li in range(L, 2 * L - 1)}

    def fused_next(li):
        return li < L

    ENGS = []

    def dyn_dma(out_ap, in_ap):
        e = ENGS.pop(0)
        ENGS.append(e)
        return e.dma_start(out_ap, in_ap)

    def free_iv(t):
        # snapped IV copies are dead once the loop closes and are safe to
        # recycle (loop_var regs are NOT safe — runtime crash)
        for hdl in t.val.handles:
            nc.free_register(hdl)

    with tile.TileContext(nc) as tc:
        ENGS.extend([nc.sync, nc.scalar])
        with (
            tc.tile_pool(name="consts", bufs=1) as consts,
            tc.tile_pool(name="wbig", bufs=1) as wbig,
            tc.tile_pool(name="wsmall", bufs=2) as wsmall,
            tc.tile_pool(name="io", bufs=3) as io,
            tc.tile_pool(name="ew1", bufs=1) as ewp1,
            tc.tile_pool(name="state", bufs=1) as state,
            tc.tile_pool(name="pgp", bufs=2, space="PSUM") as pgp,
            tc.tile_pool(name="tpp", bufs=2, space="PSUM") as tpp,
        ):
            ident = consts.tile([32, 32], f32)
            nc.sync.dma_start(ident[:], ident_dram[:])
            ident128 = consts.tile([128, 128], bf16)
            nc.sync.dma_start(ident128[:], ident128_dram[:])

            def load_w(w_dram, KC, HID, pool, tag):
                w_sb = pool.tile([128, 4 * KC * HID], bf16, tag=tag)
                for q in range(4):
                    for k in range(KC):
                        off = (q * KC + k) * HID
                        nc.sync.dma_start(w_sb[:, off:off + HID], w_dram[q, k])
                return w_sb

            def load_bias(li, HID):
                b_sb = state.tile([128, HID], f32, tag=f"bias{li % 2}")
                nc.sync.dma_start(b_sb[:], bias[li][:])
                return b_sb

            def mm_gates(pg, stat_slices, w_sb, KC, HID, n0, nw, xg_ap=None):
                """Accumulate gate pre-activations for cols [n0, n0+nw).
                stat_slices[k] is the [128,32] stationary AP for k-chunk k.
                If xg_ap is given, its [128, nw] slice seeds the PSUM through
                an identity matmul (no DVE add needed)."""
                first = xg_ap is not None
                if first:
                    nc.tensor.matmul(
                        pg[:, n0:n0 + nw], ident128[:], xg_ap,
                        start=True, stop=False, skip_group_check=True,
                    )
                for k in range(KC):
                    for q in range(4):
                        woff = (q * KC + k) * HID + n0
                        nc.tensor.matmul(
                            pg[32 * q:32 * (q + 1), n0:n0 + nw],
                            stat_slices[k],
                            w_sb[:, woff:woff + nw],
                            start=(k == 0 and not first), stop=(k == KC - 1),
                            tile_position=(0, 32 * q),
                            skip_group_check=True,
                        )

            def proj_u(pg2, stat_slices, w_sb, b_sb, KCin, HID, xgt, u):
                """projection for one sub-step into xgt pair-tile columns."""
                for (n0, nw) in _nchunks(HID):
                    mm_gates(pg2, stat_slices, w_sb, KCin, HID, n0, nw)
                    nc.vector.tensor_add(
                        xgt[:, u * HID + n0:u * HID + n0 + nw],
                        pg2[:, n0:n0 + nw], b_sb[:, n0:n0 + nw])

            def step_body(HID, pg, first, hT2, c_sb, u):
                """pg holds gate pre-activations. Compute h, write hT2[k] sub-u."""
                KC = HID // 128
                NCH = _nchunks(HID)
                tg = ewp1.tile([32, HID], f32, tag="tg")
                tch = ewp1.tile([32, HID], f32, tag="tch")
                h = ewp1.tile([32, HID], f32, tag="h")
                tp = tpp.tile([128, KC * BL], f32, tag="tp")
                for (n0, nw) in NCH:
                    nc.scalar.activation(pg[0:96, n0:n0 + nw], pg[0:96, n0:n0 + nw], Sigmoid)
                for (n0, nw) in NCH:
                    nc.scalar.activation(tg[:, n0:n0 + nw], pg[96:128, n0:n0 + nw], Tanh)
                for (n0, nw) in NCH:
                    sl = slice(n0, n0 + nw)
                    if first:
                        nc.vector.tensor_mul(c_sb[:, sl], pg[0:32, sl], tg[:, sl])
                    else:
                        m1 = ewp1.tile([32, HID], f32, tag="m1")
                        nc.vector.tensor_mul(m1[:, sl], pg[0:32, sl], tg[:, sl])
                        nc.vector.tensor_mul(c_sb[:, sl], pg[32:64, sl], c_sb[:, sl])
                        nc.vector.tensor_add(c_sb[:, sl], c_sb[:, sl], m1[:, sl])
                    nc.scalar.activation(tch[:, sl], c_sb[:, sl], Tanh)
                    nc.vector.tensor_mul(h[:, sl], pg[64:96, sl], tch[:, sl])
                    for k in range(n0 // 128, (n0 + nw) // 128):
                        nc.tensor.transpose(tp[:, 32 * k:32 * (k + 1)], h[:, 128 * k:128 * (k + 1)], ident[:])
                        nc.scalar.copy(hT2[k][:, 32 * u:32 * (u + 1)], tp[:, 32 * k:32 * (k + 1)])

            def rec(li, HID, xg_dram, hout_dram):
                KC = HID // 128
                fuse = fused_next(li) and li + 1 < len(LAYERS)
                big = HID == D
                w_sb = load_w(whh[li], KC, HID, wbig if big else wsmall, "wb" if big else "ws")
                if fuse:
                    INn, HIDn = LAYERS[li + 1]
                    wn_sb = load_w(wih[li + 1], INn // 128, HIDn, wsmall, "ws")
                    bn_sb = load_bias(li + 1, HIDn)
                    xgn_dram = xg_buf[HIDn][(li + 1) % 2]
                c_sb = state.tile([32, HID], f32, tag="c")
                hT2 = [state.tile([128, 64], bf16, tag=f"hT{k}", name=f"hT2_{k}") for k in range(KC)]

                def substeps(p_idx, pair0, xg_pair):
                    if fuse:
                        xgn = io.tile([128, 2 * HIDn], bf16, tag="xgt", bufs=2)
                    for u in (0, 1):
                        pg = pgp.tile([128, HID], f32, tag="pg")
                        stat = [hT2[k][:, 32 * (1 - u):32 * (2 - u)] for k in range(KC)]
                        for (n0, nw) in _nchunks(HID):
                            xg_ap = xg_pair[:, u * HID + n0:u * HID + n0 + nw]
                            if pair0 and u == 0:
                                nc.tensor.matmul(pg[:, n0:n0 + nw], ident128[:], xg_ap,
                                                 start=True, stop=True, skip_group_check=True)
                            else:
                                mm_gates(pg, stat, w_sb, KC, HID, n0, nw, xg_ap=xg_ap)
                        step_body(HID, pg, pair0 and u == 0, hT2, c_sb, u)
                        if fuse:
                            pgn = pgp.tile([128, HIDn], f32, tag="pg")
                            statn = [hT2[k][:, 32 * u:32 * (u + 1)] for k in range(KC)]
                            proj_u(pgn, statn, wn_sb, bn_sb, KC, HIDn, xgn, u)
                    if hout_dram is not None:
                        hout = io.tile([128, 2 * KC * BL], bf16, tag="hout", bufs=2)
                        for k in range(KC):
                            nc.vector.tensor_copy(hout[:, 64 * k:64 * (k + 1)], hT2[k][:])
                        dyn_dma(hout_dram[p_idx], hout[:])
                    if fuse:
                        dyn_dma(xgn_dram[p_idx], xgn[:])

                # pair 0 (steps 0,1) as prologue with static addressing
                g0 = io.tile([128, 2 * HID], bf16, tag="xg_in")
                nc.sync.dma_start(g0[:], xg_dram[0])
                substeps(0, True, g0)
                with tc.For_i(1, TP, staggered_reset=staggered) as t:
                    xg_t = io.tile([128, 2 * HID], bf16, tag="xg_in")
                    dyn_dma(xg_t[:], xg_dram[t])
                    substeps(t, False, xg_t)
                free_iv(t)

            def proj_loop(li, IN, HID, src_dram, xg_dram):
                KCin = IN // 128
                big = KCin == 10 and HID == D
                wp = load_w(wih[li], KCin, HID, wbig if big else wsmall, "wb" if big else "ws")
                bp = load_bias(li, HID)
                with tc.For_i(0, TP, staggered_reset=staggered) as t:
                    xin = io.tile([128, 2 * KCin * BL], bf16, tag="xin", bufs=2)
                    dyn_dma(xin[:], src_dram[t])
                    xgt = io.tile([128, 2 * HID], bf16, tag="xgt", bufs=2)
                    for u in (0, 1):
                        pg2 = pgp.tile([128, HID], f32, tag="pg")
                        stat = [xin[:, (2 * k + u) * BL:(2 * k + u + 1) * BL] for k in range(KCin)]
                        proj_u(pg2, stat, wp, bp, KCin, HID, xgt, u)
                    dyn_dma(xg_dram[t], xgt[:])
                free_iv(t)

            # enc0 projection from xT (own loop)
            proj_loop(0, D, H, xT, xg_buf[H][0])
            tc.strict_bb_all_engine_barrier()

            for li, (IN, HID) in enumerate(LAYERS):
                if li >= L + 1:
                    proj_loop(li, IN, HID, hseq_buf[li - 1], xg_buf[HID][li % 2])
                    tc.strict_bb_all_engine_barrier()
                if li == L - 1:
                    hout = enc_out
                elif li == 2 * L - 1:
                    hout = dec_out
                elif li in hseq_buf:
                    hout = hseq_buf[li]
                else:
                    hout = None
                rec(li, HID, xg_buf[HID][li % 2], hout)
                tc.strict_bb_all_engine_barrier()

    _split_waits(nc, mybir)
    return nc


def _pairify(a):
    """[T, 128, W] -> [T/2, 128, 2W]"""
    Tn, P, W = a.shape
    return a.reshape(Tn // 2, 2, P, W).transpose(0, 2, 1, 3).reshape(Tn // 2, P, 2 * W)


def prep_inputs(x, params):
    in_map_shared = {}
    for li, (W_ih, W_hh, b_ih, b_hh) in enumerate(params):
        HID = W_hh.shape[1]
        IN = W_ih.shape[1]
        perm = [0, 1, 3, 2]  # [i, f, o, g]
        wq = W_ih.reshape(4, HID, IN)[perm].transpose(0, 2, 1).reshape(4, IN // 128, 128, HID)
        in_map_shared[f"wih{li}"] = np.ascontiguousarray(wq).astype(BF)
        wq = W_hh.reshape(4, HID, HID)[perm].transpose(0, 2, 1).reshape(4, HID // 128, 128, HID)
        in_map_shared[f"whh{li}"] = np.ascontiguousarray(wq).astype(BF)
        bb = (b_ih + b_hh).reshape(4, HID)[perm]
        bb = np.repeat(bb[:, None, :], BL, axis=1).reshape(128, HID)
        in_map_shared[f"bias{li}"] = np.ascontiguousarray(bb).astype(np.float32)
    in_map_shared["ident32"] = np.eye(32, dtype=np.float32)
    in_map_shared["ident128"] = np.eye(128, dtype=np.float32).astype(BF)

    in_maps = []
    for c in range(NCORES):
        m = dict(in_map_shared)
        xc = x[:, c * BL:(c + 1) * BL, :].transpose(0, 2, 1)  # [T, D, BL]
        xc = xc.reshape(T, D // 128, 128, BL).transpose(0, 2, 1, 3)  # [T,128,KC,BL]
        xc = xc.reshape(TP, 2, 128, D // 128, BL).transpose(0, 2, 3, 1, 4)
        m["xT"] = np.ascontiguousarray(xc.reshape(TP, 128, 2 * (D // 128) * BL)).astype(BF)
        in_maps.append(m)
    return in_maps


_CACHED = {}


def kernel(x, enc_W_ih0, enc_W_ih, enc_W_hh, enc_b_ih, enc_b_hh,
           dec_W_ih0, dec_W_ih, dec_W_hh, dec_b_ih, dec_b_hh,
           _trace=False):
    from concourse.bass_utils import run_bass_kernel_spmd

    params = []
    for l in range(L):
        W_ih = enc_W_ih0 if l == 0 else enc_W_ih[l - 1]
        params.append((np.asarray(W_ih, np.float32), np.asarray(enc_W_hh[l], np.float32),
                       np.asarray(enc_b_ih[l], np.float32), np.asarray(enc_b_hh[l], np.float32)))
    for l in range(L):
        W_ih = dec_W_ih0 if l == 0 else dec_W_ih[l - 1]
        params.append((np.asarray(W_ih, np.float32), np.asarray(dec_W_hh[l], np.float32),
                       np.asarray(dec_b_ih[l], np.float32), np.asarray(dec_b_hh[l], np.float32)))

    in_maps = prep_inputs(np.asarray(x, np.float32), params)

    if "nc" not in _CACHED:
        _CACHED["nc"] = build_graph()
    nc = _CACHED["nc"]

    res = run_bass_kernel_spmd(nc, in_maps, core_ids=list(range(NCORES)), trace=_trace)
    results = res.results

    def assemble(key, HID):
        KC = HID // 128
        cores = []
        for r in results:
            a = np.asarray(r[key], dtype=np.float32)  # [TP, 128, 2*KC*BL]
            a = a.reshape(TP, 128, KC, 2, BL).transpose(0, 3, 4, 2, 1).reshape(T, BL, HID)
            cores.append(a)
        return np.concatenate(cores, axis=1)

    enc = assemble("enc_out", H)
    dec = assemble("dec_out", D)
    if _trace:
        return (enc, dec), res
    return (enc, dec)


# revision 41
# speedup vs baseline: 1.2983x; 1.2983x over previous
"""Trainium2 Bass kernel: 10-layer LSTM autoencoder (5-layer encoder H=256 +
5-layer decoder hidden=1280), T=128, B=256, D=1280.

Strategy: pure data-parallel over batch (b=32 per core, 8 cores, no
collectives).  All matmuls run with the small activation tile (32 cols) as the
PE stationary operand, replicated into the four 32-wide column groups of the
128x128 array via tile_position; each column group streams one gate-quarter of
the weights, so the array is fully utilized despite batch=32.  Gate quarters
are ordered [i, f, o, g] so a single sigmoid covers PSUM partitions 0..95.
Weights/activations are bf16 (fp32 PSUM accumulate), biases are folded into
the precomputed input projection xg.  The xg contribution enters the gates
PSUM through an identity matmul (PE) instead of a DVE add, keeping the
per-step serial chain short.

Time steps are processed in PAIRS: every sequence tensor is laid out
[T/2, 128, 2*W] so one dynamic DMA moves two steps, every For_i covers two
steps, and the h^T state is split per k-chunk into small tiles so the next
step's matmuls can start as soon as the first hidden chunk of h^T lands
(pipelining the elementwise chain under the PE).

The recurrence loop for layer l is fused with the input projection of layer
l+1 whenever both weight sets fit in SBUF (all encoder boundaries + dec0);
decoder-to-decoder boundaries run as separate projection passes through DRAM.
Engine registers are the scarce resource (2 per For_i per engine + 2 per
dynamic-DMA site, and only snapped IV copies can be safely recycled), so
dynamic DMAs alternate between the two HWDGE-capable queues (SP, Act).
"""
import sys
import os
import numpy as np
import ml_dtypes

sys.path.insert(0, "/opt/trn_rl_repo")

T, B, D, H, L = 128, 256, 1280, 256, 5
NCORES = 8
BL = B // NCORES  # 32 batch per core
TP = T // 2       # time pairs

LAYERS = [(D, H)] + [(H, H)] * 4 + [(H, D)] + [(D, D)] * 4

BF = ml_dtypes.bfloat16


def _nchunks(hid):
    out = []
    n0 = 0
    while n0 < hid:
        out.append((n0, min(512, hid - n0)))
        n0 += 512
    return out


def _split_waits(nc, mybir):
    """Walrus's per-opcode instruction structs hold very few sync-wait slots
    (LDWEIGHTS holds one).  Tile can emit many waits on one instruction, so
    dedupe same-semaphore waits (keep max threshold) and spill all but one
    onto dedicated NoOps placed immediately before the instruction."""
    counter = [0]
    for fn in nc.m.functions:
        for bb in fn.blocks:
            out = []
            for ins in bb.instructions:
                si = getattr(ins, "sync_info", None)
                if si is not None and si.on_wait and len(si.on_wait) > 1:
                    merged = {}
                    order = []
                    for w in si.on_wait:
                        key = (w.sync_type, w.id, w.wait_mode, str(w.wait_reg))
                        if key in merged:
                            old = merged[key]
                            if (w.wait_value is not None and old.wait_value is not None
                                    and w.wait_value > old.wait_value):
                                merged[key] = w
                        else:
                            merged[key] = w
                            order.append(key)
                    waits = [merged[k] for k in order]
                    for w in waits[:-1]:
                        counter[0] += 1
                        out.append(mybir.InstNoOp(
                            name=f"waitspill-{counter[0]}",
                            engine=ins.engine,
                            ins=[], outs=[],
                            sync_info=mybir.SyncInfo(on_wait=[w], on_update=[]),
                        ))
                    ins.sync_info = mybir.SyncInfo(on_wait=[waits[-1]], on_update=si.on_update)
                out.append(ins)
            bb.instructions = out


def build_graph(staggered=None):
    from concourse import bass, tile
    try:
        from concourse import tile_utils
        tile_utils.max_sbuf_usage = 206 * 1024
    except Exception:
        pass
    import concourse.mybir as mybir

    if staggered is None:
        staggered = os.environ.get("STAGGERED", "0") == "1"

    f32 = mybir.dt.float32
    bf16 = mybir.dt.bfloat16
    Sigmoid = mybir.ActivationFunctionType.Sigmoid
    Tanh = mybir.ActivationFunctionType.Tanh

    nc = bass.Bass()

    xT = nc.declare_dram_parameter("xT", [TP, 128, 2 * (D // 128) * BL], bf16, isOutput=False)
    wih, whh, bias = [], [], []
    for li, (IN, HID) in enumerate(LAYERS):
        wih.append(nc.declare_dram_parameter(f"wih{li}", [4, IN // 128, 128, HID], bf16, isOutput=False))
        whh.append(nc.declare_dram_parameter(f"whh{li}", [4, HID // 128, 128, HID], bf16, isOutput=False))
        bias.append(nc.declare_dram_parameter(f"bias{li}", [128, HID], f32, isOutput=False))
    ident_dram = nc.declare_dram_parameter("ident32", [32, 32], f32, isOutput=False)
    ident128_dram = nc.declare_dram_parameter("ident128", [128, 128], bf16, isOutput=False)
    enc_out = nc.declare_dram_parameter("enc_out", [TP, 128, 2 * (H // 128) * BL], bf16, isOutput=True)
    dec_out = nc.declare_dram_parameter("dec_out", [TP, 128, 2 * (D // 128) * BL], bf16, isOutput=True)

    xg_buf = {
        hid: [nc.dram_tensor(f"xg{hid}_{i}", [TP, 128, 2 * hid], bf16) for i in range(2)]
        for hid in (H, D)
    }
    hseq_buf = {li: nc.dram_tensor(f"hseq{li}", [TP, 128, 2 * (D // 128) * BL], bf16)
                for li in range(L, 2 * L - 1)}

    def fused_next(li):
        return li < L

    ENGS = []

    def dyn_dma(out_ap, in_ap):
        e = ENGS.pop(0)
        ENGS.append(e)
        return e.dma_start(out_ap, in_ap)

    def free_iv(t):
        # snapped IV copies are dead once the loop closes and are safe to
        # recycle (loop_var regs are NOT safe — runtime crash)
        for hdl in t.val.handles:
            nc.free_register(hdl)

    with tile.TileContext(nc) as tc:
        ENGS.extend([nc.sync, nc.scalar])
        with (
            tc.tile_pool(name="consts", bufs=1) as consts,
            tc.tile_pool(name="wbig", bufs=1) as wbig,
            tc.tile_pool(name="wsmall", bufs=2) as wsmall,
            tc.tile_pool(name="io", bufs=3) as io,
            tc.tile_pool(name="ew1", bufs=1) as ewp1,
            tc.tile_pool(name="state", bufs=1) as state,
            tc.tile_pool(name="pgp", bufs=2, space="PSUM") as pgp,
            tc.tile_pool(name="tpp", bufs=2, space="PSUM") as tpp,
        ):
            ident = consts.tile([32, 32], f32)
            nc.sync.dma_start(ident[:], ident_dram[:])
            ident128 = consts.tile([128, 128], bf16)
            nc.sync.dma_start(ident128[:], ident128_dram[:])

            def load_w(w_dram, KC, HID, pool, tag):
                w_sb = pool.tile([128, 4 * KC * HID], bf16, tag=tag)
                for q in range(4):
                    for k in range(KC):
                        off = (q * KC + k) * HID
                        nc.sync.dma_start(w_sb[:, off:off + HID], w_dram[q, k])
                return w_sb

            def load_bias(li, HID):
                b_sb = state.tile([128, HID], f32, tag=f"bias{li % 2}")
                nc.sync.dma_start(b_sb[:], bias[li][:])
                return b_sb

            def mm_gates(pgc, stat_slices, w_sb, KC, HID, n0, nw, xg_ap=None):
                """Accumulate gate pre-activations for global cols [n0, n0+nw)
                into the chunk-local PSUM tile pgc (cols 0..nw).  A chunk-local
                tile per 512-col slice lets the elementwise chain of chunk i
                run while the PE still streams chunk i+1."""
                first = xg_ap is not None
                if first:
                    nc.tensor.matmul(
                        pgc[:, 0:nw], ident128[:], xg_ap,
                        start=True, stop=False, skip_group_check=True,
                    )
                for k in range(KC):
                    for q in range(4):
                        woff = (q * KC + k) * HID + n0
                        nc.tensor.matmul(
                            pgc[32 * q:32 * (q + 1), 0:nw],
                            stat_slices[k],
                            w_sb[:, woff:woff + nw],
                            start=(k == 0 and not first), stop=(k == KC - 1),
                            tile_position=(0, 32 * q),
                            skip_group_check=True,
                        )

            def proj_u(stat_slices, w_sb, b_sb, KCin, HID, xgt, u):
                """projection for one sub-step into xgt pair-tile columns."""
                for ci, (n0, nw) in enumerate(_nchunks(HID)):
                    pgc = pgp.tile([128, nw], f32, tag=f"pg{ci}", name=f"pgp_{ci}")
                    mm_gates(pgc, stat_slices, w_sb, KCin, HID, n0, nw)
                    nc.vector.tensor_add(
                        xgt[:, u * HID + n0:u * HID + n0 + nw],
                        pgc[:, 0:nw], b_sb[:, n0:n0 + nw])

            def step_body(HID, pgcs, first, hT2, c_sb, u):
                """pgcs: per-chunk gate PSUM tiles. Compute h, write hT2[k] sub-u."""
                KC = HID // 128
                NCH = _nchunks(HID)
                tg = ewp1.tile([32, HID], f32, tag="tg")
                tch = ewp1.tile([32, HID], f32, tag="tch")
                h = ewp1.tile([32, HID], f32, tag="h")
                tp = tpp.tile([128, KC * BL], f32, tag="tp")
                for ci, (n0, nw) in enumerate(NCH):
                    pgc = pgcs[ci]
                    sl = slice(n0, n0 + nw)
                    lw = slice(0, nw)
                    nc.scalar.activation(pgc[0:96, lw], pgc[0:96, lw], Sigmoid)
                    nc.scalar.activation(tg[:, sl], pgc[96:128, lw], Tanh)
                    if first:
                        nc.vector.tensor_mul(c_sb[:, sl], pgc[0:32, lw], tg[:, sl])
                    else:
                        m1 = ewp1.tile([32, HID], f32, tag="m1")
                        nc.vector.tensor_mul(m1[:, sl], pgc[0:32, lw], tg[:, sl])
                        nc.vector.tensor_mul(c_sb[:, sl], pgc[32:64, lw], c_sb[:, sl])
                        nc.vector.tensor_add(c_sb[:, sl], c_sb[:, sl], m1[:, sl])
                    nc.scalar.activation(tch[:, sl], c_sb[:, sl], Tanh)
                    nc.vector.tensor_mul(h[:, sl], pgc[64:96, lw], tch[:, sl])
                    for k in range(n0 // 128, (n0 + nw) // 128):
                        nc.tensor.transpose(tp[:, 32 * k:32 * (k + 1)], h[:, 128 * k:128 * (k + 1)], ident[:])
                        nc.scalar.copy(hT2[k][:, 32 * u:32 * (u + 1)], tp[:, 32 * k:32 * (k + 1)])

            def rec(li, HID, xg_dram, hout_dram):
                KC = HID // 128
                fuse = fused_next(li) and li + 1 < len(LAYERS)
                big = HID == D
                w_sb = load_w(whh[li], KC, HID, wbig if big else wsmall, "wb" if big else "ws")
                if fuse:
                    INn, HIDn = LAYERS[li + 1]
                    wn_sb = load_w(wih[li + 1], INn // 128, HIDn, wsmall, "ws")
                    bn_sb = load_bias(li + 1, HIDn)
                    xgn_dram = xg_buf[HIDn][(li + 1) % 2]
                c_sb = state.tile([32, HID], f32, tag="c")
                hT2 = [state.tile([128, 64], bf16, tag=f"hT{k}", name=f"hT2_{k}") for k in range(KC)]

                def substeps(p_idx, pair0, xg_pair):
                    if fuse:
                        xgn = io.tile([128, 2 * HIDn], bf16, tag="xgt", bufs=2)
                    for u in (0, 1):
                        stat = [hT2[k][:, 32 * (1 - u):32 * (2 - u)] for k in range(KC)]
                        pgcs = []
                        for ci, (n0, nw) in enumerate(_nchunks(HID)):
                            pgc = pgp.tile([128, nw], f32, tag=f"pg{ci}", name=f"pgr_{ci}")
                            pgcs.append(pgc)
                            xg_ap = xg_pair[:, u * HID + n0:u * HID + n0 + nw]
                            if pair0 and u == 0:
                                nc.tensor.matmul(pgc[:, 0:nw], ident128[:], xg_ap,
                                                 start=True, stop=True, skip_group_check=True)
                            else:
                                mm_gates(pgc, stat, w_sb, KC, HID, n0, nw, xg_ap=xg_ap)
                        step_body(HID, pgcs, pair0 and u == 0, hT2, c_sb, u)
                        if fuse:
                            statn = [hT2[k][:, 32 * u:32 * (u + 1)] for k in range(KC)]
                            proj_u(statn, wn_sb, bn_sb, KC, HIDn, xgn, u)
                    if hout_dram is not None:
                        hout = io.tile([128, 2 * KC * BL], bf16, tag="hout", bufs=2)
                        for k in range(KC):
                            nc.vector.tensor_copy(hout[:, 64 * k:64 * (k + 1)], hT2[k][:])
                        dyn_dma(hout_dram[p_idx], hout[:])
                    if fuse:
                        dyn_dma(xgn_dram[p_idx], xgn[:])

                # pair 0 (steps 0,1) as prologue with static addressing
                g0 = io.tile([128, 2 * HID], bf16, tag="xg_in")
                nc.sync.dma_start(g0[:], xg_dram[0])
                substeps(0, True, g0)
                with tc.For_i(1, TP, staggered_reset=staggered) as t:
                    xg_t = io.tile([128, 2 * HID], bf16, tag="xg_in")
                    dyn_dma(xg_t[:], xg_dram[t])
                    substeps(t, False, xg_t)
                free_iv(t)

            def proj_loop(li, IN, HID, src_dram, xg_dram):
                KCin = IN // 128
                big = KCin == 10 and HID == D
                wp = load_w(wih[li], KCin, HID, wbig if big else wsmall, "wb" if big else "ws")
                bp = load_bias(li, HID)
                with tc.For_i(0, TP, staggered_reset=staggered) as t:
                    xin = io.tile([128, 2 * KCin * BL], bf16, tag="xin", bufs=2)
                    dyn_dma(xin[:], src_dram[t])
                    xgt = io.tile([128, 2 * HID], bf16, tag="xgt", bufs=2)
                    for u in (0, 1):
                        stat = [xin[:, (2 * k + u) * BL:(2 * k + u + 1) * BL] for k in range(KCin)]
                        proj_u(stat, wp, bp, KCin, HID, xgt, u)
                    dyn_dma(xg_dram[t], xgt[:])
                free_iv(t)

            # enc0 projection from xT (own loop)
            proj_loop(0, D, H, xT, xg_buf[H][0])
            tc.strict_bb_all_engine_barrier()

            for li, (IN, HID) in enumerate(LAYERS):
                if li >= L + 1:
                    proj_loop(li, IN, HID, hseq_buf[li - 1], xg_buf[HID][li % 2])
                    tc.strict_bb_all_engine_barrier()
                if li == L - 1:
                    hout = enc_out
                elif li == 2 * L - 1:
                    hout = dec_out
                elif li in hseq_buf:
                    hout = hseq_buf[li]
                else:
                    hout = None
                rec(li, HID, xg_buf[HID][li % 2], hout)
                tc.strict_bb_all_engine_barrier()

    _split_waits(nc, mybir)
    return nc


def _pairify(a):
    """[T, 128, W] -> [T/2, 128, 2W]"""
    Tn, P, W = a.shape
    return a.reshape(Tn // 2, 2, P, W).transpose(0, 2, 1, 3).reshape(Tn // 2, P, 2 * W)


def prep_inputs(x, params):
    in_map_shared = {}
    for li, (W_ih, W_hh, b_ih, b_hh) in enumerate(params):
        HID = W_hh.shape[1]
        IN = W_ih.shape[1]
        perm = [0, 1, 3, 2]  # [i, f, o, g]
        wq = W_ih.reshape(4, HID, IN)[perm].transpose(0, 2, 1).reshape(4, IN // 128, 128, HID)
        in_map_shared[f"wih{li}"] = np.ascontiguousarray(wq).astype(BF)
        wq = W_hh.reshape(4, HID, HID)[perm].transpose(0, 2, 1).reshape(4, HID // 128, 128, HID)
        in_map_shared[f"whh{li}"] = np.ascontiguousarray(wq).astype(BF)
        bb = (b_ih + b_hh).reshape(4, HID)[perm]
        bb = np.repeat(bb[:, None, :], BL, axis=1).reshape(128, HID)
        in_map_shared[f"bias{li}"] = np.ascontiguousarray(bb).astype(np.float32)
    in_map_shared["ident32"] = np.eye(32, dtype=np.float32)
    in_map_shared["ident128"] = np.eye(128, dtype=np.float32).astype(BF)

    in_maps = []
    for c in range(NCORES):
        m = dict(in_map_shared)
        xc = x[:, c * BL:(c + 1) * BL, :].transpose(0, 2, 1)  # [T, D, BL]
        xc = xc.reshape(T, D // 128, 128, BL).transpose(0, 2, 1, 3)  # [T,128,KC,BL]
        xc = xc.reshape(TP, 2, 128, D // 128, BL).transpose(0, 2, 3, 1, 4)
        m["xT"] = np.ascontiguousarray(xc.reshape(TP, 128, 2 * (D // 128) * BL)).astype(BF)
        in_maps.append(m)
    return in_maps


_CACHED = {}


def kernel(x, enc_W_ih0, enc_W_ih, enc_W_hh, enc_b_ih, enc_b_hh,
           dec_W_ih0, dec_W_ih, dec_W_hh, dec_b_ih, dec_b_hh,
           _trace=False):
    from concourse.bass_utils import run_bass_kernel_spmd

    params = []
    for l in range(L):
        W_ih = enc_W_ih0 if l == 0 else enc_W_ih[l - 1]
        params.append((np.asarray(W_ih, np.float32), np.asarray(enc_W_hh[l], np.float32),
                       np.asarray(enc_b_ih[l], np.float32), np.asarray(enc_b_hh[l], np.float32)))
    for l in range(L):
        W_ih = dec_W_ih0 if l == 0 else dec_W_ih[l - 1]
        params.append((np.asarray(W_ih, np.float32), np.asarray(dec_W_hh[l], np.float32),
                       np.asarray(dec_b_ih[l], np.float32), np.asarray(dec_b_hh[l], np.float32)))

    in_maps = prep_inputs(np.asarray(x, np.float32), params)

    if "nc" not in _CACHED:
        _CACHED["nc"] = build_graph()
    nc = _CACHED["nc"]

    res = run_bass_kernel_spmd(nc, in_maps, core_ids=list(range(NCORES)), trace=_trace)
    results = res.results

    def assemble(key, HID):
        KC = HID // 128
        cores = []
        for r in results:
            a = np.asarray(r[key], dtype=np.float32)  # [TP, 128, 2*KC*BL]
            a = a.reshape(TP, 128, KC, 2, BL).transpose(0, 3, 4, 2, 1).reshape(T, BL, HID)
            cores.append(a)
        return np.concatenate(cores, axis=1)

    enc = assemble("enc_out", H)
    dec = assemble("dec_out", D)
    if _trace:
        return (enc, dec), res
    return (enc, dec)
